# revision 5
# baseline (speedup 1.0000x reference)
"""Trainium2 Bass kernel for nn_Attention_75342316306884.

Per-batch channel-channel attention:
  xf = x.reshape(B, C, HW); cf = condition.reshape(B, C, HW)
  w1 = softmax(xf @ xf^T * HW^-0.5); w2 = softmax(sig(cf) @ sig(cf)^T * HW^-0.5)
  out = xf + (w1 + w2) @ xf          -> [B, C, HW] float32

Sharding: pure data parallel, batch dim 64 -> 8 cores x 8 batches.

Per-core pipeline, software-skewed two batches ahead (emission order
stage(b+2), gram(b+1), apply(b)):
  stage: one cast-DMA per tensor (f32 HBM -> bf16 [128, 4, 896] SBUF tile;
    pad cols zeroed by a gpsimd memset so gram chunks can run K=128 over
    the pad); condition -> sigmoid via tanh; ONE SBUF->SBUF xbar
    DMA-transpose per tensor into a [128, 28, 128] bf16 tile.
  gram: two 512x512 grams on TensorE (bf16, f32 PSUM accumulate), 7 full
    K=128 chunk matmuls each (zero pad contributes nothing); ACT exp with
    fused per-row accumulation into one packed z [128, 8] tile.
  combine (the main trick vs the two-chain baseline):
    r = 1/z on DVE; the merged attention matrix
    W[c, d] = r1[c]*E1[c,d] + r2[c]*E2[c,d] (+ I for the residual) is
    built in the native [c-part, d-free] layout with per-partition
    scalar ops (2 DVE ops per c-block: tensor_scalar mult +
    scalar_tensor_tensor fused mult-add), then ONE SBUF->SBUF xbar
    DMA-transpose turns it into the [d-part, c-free] stationary operand
    the apply needs.
  apply: ONE matmul chain per c-block (4 K=128 chunks x 2 n-splits) into a
    2-bank [128, 1024] PSUM tile; epilogue is a single PSUM->SBUF bf16
    copy (alternating ACT/DVE) and one cast-DMA per batch back to f32 HBM.

vs. the two-chain version this halves apply TensorE work (no separate
E2 chain, no diag-residual chain, no per-chain scale/add epilogue) at the
cost of ~12 cheap DVE/gpsimd ops per batch.  exp without max-subtraction
is safe: logits bounded by ~|x|^2/28 ~ 35.
"""

import sys

import numpy as np

for _p in ("/opt/trn_rl_repo",):
    if _p not in sys.path:
        sys.path.append(_p)

import ml_dtypes

import concourse.bass as bass
import concourse.mybir as mybir
import concourse.tile as tile
from concourse.bass_utils import run_bass_kernel_spmd
from concourse.vector_clock import ScopedClock

F32 = mybir.dt.float32
BF16 = mybir.dt.bfloat16
AF = mybir.ActivationFunctionType
MUL = mybir.AluOpType.mult
ADD = mybir.AluOpType.add

N_CORES = 8
B_PER_CORE = 8
C = 512  # channels
HW = 784  # 28*28
HWP = 896  # padded to 7*128 for the xbar transpose
SCALE = float(HW) ** -0.5
P = 128
N_KCH = 7  # gram contraction chunks, all K=128 (pad zeroed)
N_CB = 4  # 512 / 128 c-blocks
APPLY_NSPLIT = ((0, 512), (512, 272))


def _patch_tile_drain():
    """walrus codegen in this toolchain rejects >1 sem-wait on one SP CTRL
    (drain/nop) instruction; spread the Tile end-of-context drain waits
    across several nops instead."""
    if getattr(tile.TileContext, "_drain_patched", False):
        return

    def _drain_and_barrier(self, tick_clock, wait_clock):
        absorber = self.nc.sync.nop()
        wait_clock.add_sem_waits(
            absorber.ins, ScopedClock({None: tick_clock.global_clock})
        )
        si = absorber.ins.sync_info
        waits = list(si.on_wait) if si is not None and si.on_wait else []
        if len(waits) > 1:
            absorber.ins.sync_info = mybir.SyncInfo(on_wait=waits[:1], on_update=[])
            for w in waits[1:]:
                n2 = self.nc.sync.nop()
                n2.ins.sync_info = mybir.SyncInfo(on_wait=[w], on_update=[])
        self.nc.sync.drain()
        self.nc.all_engine_barrier()
        assert self.sems is not None
        popped = self.nc._tile_sem_poison_stack.pop()
        assert popped is self._sem_poison
        self.nc.clear_and_free_semaphores(list(self.sems.allocated().values()))
        self.nc.all_engine_barrier()

    tile.TileContext._drain_and_barrier = _drain_and_barrier
    tile.TileContext._drain_patched = True


def _split_multi_waits(nc, limit=1):
    """This walrus build allows only `limit` sem-wait commands per
    instruction.  Hoist excess waits onto same-engine NoOps placed
    immediately before the instruction (per-engine program order makes
    this semantically identical)."""
    n_split = 0
    for f in nc.m.functions:
        for bb in f.blocks:
            new_insts = []
            for inst in bb.instructions:
                si = inst.sync_info
                waits = list(si.on_wait) if si is not None and si.on_wait else []
                if len(waits) > limit:
                    for j, w in enumerate(waits[:-limit]):
                        nop = mybir.InstNoOp(
                            name=f"{inst.name}-wsplit{j}", ins=[], outs=[]
                        )
                        nop.engine = inst.engine
                        nop.sync_info = mybir.SyncInfo(on_wait=[w], on_update=[])
                        new_insts.append(nop)
                    inst.sync_info = mybir.SyncInfo(
                        on_wait=waits[-limit:],
                        on_update=list(si.on_update) if si.on_update else [],
                    )
                    n_split += 1
                new_insts.append(inst)
            if len(new_insts) != len(bb.instructions):
                bb.instructions = new_insts
                assert len(bb.instructions) == len(new_insts)
    return n_split


def _gram_exp(nc, psum_g, opT, e_pool, z_pack, zoff, etag):
    """opT: one [128, 28, 128] bf16 transposed tile (pad rows zero).
    Returns E = exp(scale*gram) (4 x [128, 512] bf16); rowsum(E) goes to
    z_pack[:, zoff+cb] f32 via the fused ACT accumulator."""
    es = []
    for cb in range(N_CB):
        g = psum_g.tile([P, C], F32, tag="g")
        for k in range(N_KCH):
            nc.tensor.matmul(
                g[:],
                opT[:, N_KCH * cb + k, :],
                opT[:, k :: N_KCH, :],
                start=(k == 0),
                stop=(k == N_KCH - 1),
            )
        e = e_pool.tile([P, C], BF16, tag=etag)
        nc.scalar.activation(
            e[:],
            g[:],
            AF.Exp,
            scale=SCALE,
            accum_out=z_pack[:, zoff + cb : zoff + cb + 1],
        )
        es.append(e)
    return es


def build_kernel():
    _patch_tile_drain()
    nc = bass.Bass()
    x_ext = nc.declare_dram_parameter("x", [B_PER_CORE, C, HW], F32, isOutput=False)
    c_ext = nc.declare_dram_parameter(
        "condition", [B_PER_CORE, C, HW], F32, isOutput=False
    )
    out_ext = nc.declare_dram_parameter("out", [B_PER_CORE, C, HW], F32, isOutput=True)

    eye_dram = nc.inline_tensor(np.eye(P, dtype=ml_dtypes.bfloat16), name="eye128")

    with tile.TileContext(nc) as tc:
        with (
            tc.tile_pool(name="const", bufs=1) as const_pool,
            tc.tile_pool(name="xn", bufs=3) as xn_pool,
            tc.tile_pool(name="cs", bufs=2) as cs_pool,
            tc.tile_pool(name="ct", bufs=2) as ct_pool,
            tc.tile_pool(name="xT", bufs=3) as xT_pool,
            tc.tile_pool(name="csT", bufs=3) as csT_pool,
            tc.tile_pool(name="E", bufs=12) as e_pool,
            tc.tile_pool(name="W", bufs=2) as w_pool,
            tc.tile_pool(name="WT", bufs=3) as wT_pool,
            tc.tile_pool(name="Wt", bufs=2) as t_pool,
            tc.tile_pool(name="zr", bufs=6) as zr_pool,
            tc.tile_pool(name="outs", bufs=2) as out_pool,
            tc.tile_pool(name="psum_g", bufs=2, space="PSUM") as psum_g,
            tc.tile_pool(name="psum_u", bufs=2, space="PSUM") as psum_u,
        ):
            eye = const_pool.tile([P, P], BF16)
            nc.sync.dma_start(eye[:], eye_dram[:])

            staged = {}
            grams = {}

            def stage(b):
                """cast-loads + sigmoid-via-tanh + SBUF->SBUF xbar transposes."""
                xTb = xT_pool.tile([P, N_CB * N_KCH, P], BF16, tag="xT")
                csTb = csT_pool.tile([P, N_CB * N_KCH, P], BF16, tag="csT")
                xnb = xn_pool.tile([P, N_CB, HWP], BF16, tag="xn")
                nc.gpsimd.memset(xnb[:, :, HW:], 0.0)
                nc.gpsimd.dma_start(
                    xnb[:, :, :HW],
                    x_ext[b].rearrange("(k p) n -> p k n", p=P),
                )
                nc.sync.dma_start_transpose(xTb[:], xnb.rearrange("p k n -> p (k n)"))
                csb = cs_pool.tile([P, N_CB, HWP], BF16, tag="cs")
                ct = ct_pool.tile([P, N_CB, HW], BF16, tag="ct")
                nc.gpsimd.memset(csb[:, :, HW:], 0.0)
                nc.gpsimd.dma_start(
                    csb[:, :, :HW], c_ext[b].rearrange("(k p) n -> p k n", p=P)
                )
                # sigmoid(z) = 0.5 * tanh(z/2) + 0.5 (same ACT set as exp)
                nc.scalar.activation(ct[:], csb[:, :, :HW], AF.Tanh, scale=0.5)
                nc.vector.tensor_scalar(csb[:, :, :HW], ct[:], 0.5, 0.5, MUL, ADD)
                nc.sync.dma_start_transpose(csTb[:], csb.rearrange("p k n -> p (k n)"))
                staged[b] = (xnb, xTb, csTb)

            def gram_stage(b):
                xnb, xTb, csTb = staged.pop(b)
                z_pack = zr_pool.tile([P, 8], F32, tag="z")
                E1 = _gram_exp(nc, psum_g, xTb, e_pool, z_pack, 0, "e1")
                E2 = _gram_exp(nc, psum_g, csTb, e_pool, z_pack, 4, "e2")

                # W[c,d] = r1[c]*E1[c,d] + r2[c]*E2[c,d] (+I on diag block),
                # built per-partition in [c-part, d-free] layout, then one
                # xbar transpose to the [d-part, c-free] stationary layout.
                r_pack = zr_pool.tile([P, 8], F32, tag="r")
                nc.vector.reciprocal(r_pack[:], z_pack[:])
                wbig = w_pool.tile([P, N_CB, C], BF16, tag="w")
                for a in range(N_CB):
                    t1 = t_pool.tile([P, C], BF16, tag="t1")
                    nc.vector.tensor_scalar(
                        t1[:], E1[a][:], r_pack[:, a : a + 1], None, MUL
                    )
                    nc.vector.scalar_tensor_tensor(
                        wbig[:, a, :],
                        E2[a][:],
                        r_pack[:, 4 + a : 5 + a],
                        t1[:],
                        MUL,
                        ADD,
                    )
                    nc.vector.tensor_add(
                        wbig[:, a, a * P : (a + 1) * P],
                        wbig[:, a, a * P : (a + 1) * P],
                        eye[:],
                    )
                WT = wT_pool.tile([P, N_CB * N_CB, P], BF16, tag="wT")
                nc.sync.dma_start_transpose(
                    WT[:], wbig.rearrange("p a d -> p (a d)")
                )
                grams[b] = (xnb, WT)

            def apply_stage(b):
                xnb, WT = grams.pop(b)
                obig = out_pool.tile([P, N_CB, HW], BF16, tag="o")
                for cb in range(N_CB):
                    u = psum_u.tile([P, 1024], F32, tag="u")
                    for n0, nw in APPLY_NSPLIT:
                        for k in range(N_CB):
                            nc.tensor.matmul(
                                u[:, n0 : n0 + nw],
                                WT[:, N_CB * cb + k, :],
                                xnb[:, k, n0 : n0 + nw],
                                start=(k == 0),
                                stop=(k == N_CB - 1),
                            )
                    if cb % 2 == 0:
                        nc.scalar.activation(obig[:, cb, :], u[:, :HW], AF.Copy)
                    else:
                        nc.vector.tensor_copy(obig[:, cb, :], u[:, :HW])
                nc.gpsimd.dma_start(
                    out_ext[b].rearrange("(k p) n -> p k n", p=P), obig[:]
                )

            stage(0)
            stage(1)
            gram_stage(0)
            for b in range(B_PER_CORE):
                if b + 2 < B_PER_CORE:
                    stage(b + 2)
                if b + 1 < B_PER_CORE:
                    gram_stage(b + 1)
                apply_stage(b)
    n = _split_multi_waits(nc)
    print(f"[kernel] split {n} multi-wait instructions")
    return nc


_NC_CACHE = None


def kernel(x: np.ndarray, condition: np.ndarray, _trace: bool = False):
    """Full inputs [64, 512, 28, 28] f32 -> full output [64, 512, 784] f32."""
    global _NC_CACHE
    B = x.shape[0]
    xf = np.ascontiguousarray(x.reshape(B, C, HW), dtype=np.float32)
    cf = np.ascontiguousarray(condition.reshape(B, C, HW), dtype=np.float32)

    if _NC_CACHE is None:
        _NC_CACHE = build_kernel()
    nc = _NC_CACHE

    in_maps = [
        {
            "x": xf[i * B_PER_CORE : (i + 1) * B_PER_CORE],
            "condition": cf[i * B_PER_CORE : (i + 1) * B_PER_CORE],
        }
        for i in range(N_CORES)
    ]
    res = run_bass_kernel_spmd(nc, in_maps, core_ids=list(range(N_CORES)), trace=_trace)
    out = np.concatenate([res.results[i]["out"] for i in range(N_CORES)], axis=0)
    kernel.last_result = res
    return out


# revision 7
# speedup vs baseline: 1.1441x; 1.1441x over previous
"""Trainium2 Bass kernel for nn_Attention_75342316306884.

Per-batch channel-channel attention:
  xf = x.reshape(B, C, HW); cf = condition.reshape(B, C, HW)
  w1 = softmax(xf @ xf^T * HW^-0.5); w2 = softmax(sig(cf) @ sig(cf)^T * HW^-0.5)
  out = xf + (w1 + w2) @ xf          -> [B, C, HW] float32

Sharding: pure data parallel, batch dim 64 -> 8 cores x 8 batches.

Per-core pipeline, software-skewed two batches ahead (emission order
stage(b+2), gram(b+1), apply(b)):
  stage: one cast-DMA per tensor (f32 HBM -> bf16 [128, 4, 896] SBUF tile;
    pad cols zeroed by a gpsimd memset so gram chunks can run K=128 over
    the pad); condition -> sigmoid via tanh; ONE SBUF->SBUF xbar
    DMA-transpose per tensor into a [128, 28, 128] bf16 tile.
  gram: two 512x512 grams on TensorE (bf16, f32 PSUM accumulate), 7 full
    K=128 chunk matmuls each (zero pad contributes nothing); ACT exp with
    fused per-row accumulation into one packed z [128, 8] tile.
  combine (the main trick vs the two-chain baseline):
    r = 1/z on DVE; the merged attention matrix
    W[c, d] = r1[c]*E1[c,d] + r2[c]*E2[c,d] (+ I for the residual) is
    built in the native [c-part, d-free] layout with per-partition
    scalar ops (2 DVE ops per c-block: tensor_scalar mult +
    scalar_tensor_tensor fused mult-add), then 16 TensorE [128,128] block
    transposes (+ ACT/DVE PSUM->SBUF copies) turn it into the
    [d-part, c-free] stationary operand the apply needs.  The x/cs xbar
    DMA-transposes are split into 4 per-c-block instructions each so
    their descriptor streams interleave across DMA queues instead of
    serializing behind one another on the sync engine.
  apply: ONE matmul chain per c-block (4 K=128 chunks x 2 n-splits) into a
    2-bank [128, 1024] PSUM tile; epilogue is a single PSUM->SBUF bf16
    copy (alternating ACT/DVE) and one cast-DMA per batch back to f32 HBM.

vs. the two-chain version this halves apply TensorE work (no separate
E2 chain, no diag-residual chain, no per-chain scale/add epilogue) at the
cost of ~12 cheap DVE/gpsimd ops per batch.  exp without max-subtraction
is safe: logits bounded by ~|x|^2/28 ~ 35.
"""

import sys

import numpy as np

for _p in ("/opt/trn_rl_repo",):
    if _p not in sys.path:
        sys.path.append(_p)

import ml_dtypes

import concourse.bass as bass
import concourse.mybir as mybir
import concourse.tile as tile
from concourse.bass_utils import run_bass_kernel_spmd
from concourse.vector_clock import ScopedClock

F32 = mybir.dt.float32
BF16 = mybir.dt.bfloat16
AF = mybir.ActivationFunctionType
MUL = mybir.AluOpType.mult
ADD = mybir.AluOpType.add

N_CORES = 8
B_PER_CORE = 8
C = 512  # channels
HW = 784  # 28*28
HWP = 896  # padded to 7*128 for the xbar transpose
SCALE = float(HW) ** -0.5
P = 128
N_KCH = 7  # gram contraction chunks, all K=128 (pad zeroed)
N_CB = 4  # 512 / 128 c-blocks
APPLY_NSPLIT = ((0, 512), (512, 272))


def _patch_tile_drain():
    """walrus codegen in this toolchain rejects >1 sem-wait on one SP CTRL
    (drain/nop) instruction; spread the Tile end-of-context drain waits
    across several nops instead."""
    if getattr(tile.TileContext, "_drain_patched", False):
        return

    def _drain_and_barrier(self, tick_clock, wait_clock):
        absorber = self.nc.sync.nop()
        wait_clock.add_sem_waits(
            absorber.ins, ScopedClock({None: tick_clock.global_clock})
        )
        si = absorber.ins.sync_info
        waits = list(si.on_wait) if si is not None and si.on_wait else []
        if len(waits) > 1:
            absorber.ins.sync_info = mybir.SyncInfo(on_wait=waits[:1], on_update=[])
            for w in waits[1:]:
                n2 = self.nc.sync.nop()
                n2.ins.sync_info = mybir.SyncInfo(on_wait=[w], on_update=[])
        self.nc.sync.drain()
        self.nc.all_engine_barrier()
        assert self.sems is not None
        popped = self.nc._tile_sem_poison_stack.pop()
        assert popped is self._sem_poison
        self.nc.clear_and_free_semaphores(list(self.sems.allocated().values()))
        self.nc.all_engine_barrier()

    tile.TileContext._drain_and_barrier = _drain_and_barrier
    tile.TileContext._drain_patched = True


def _split_multi_waits(nc, limit=1):
    """This walrus build allows only `limit` sem-wait commands per
    instruction.  Hoist excess waits onto same-engine NoOps placed
    immediately before the instruction (per-engine program order makes
    this semantically identical)."""
    n_split = 0
    for f in nc.m.functions:
        for bb in f.blocks:
            new_insts = []
            for inst in bb.instructions:
                si = inst.sync_info
                waits = list(si.on_wait) if si is not None and si.on_wait else []
                if len(waits) > limit:
                    for j, w in enumerate(waits[:-limit]):
                        nop = mybir.InstNoOp(
                            name=f"{inst.name}-wsplit{j}", ins=[], outs=[]
                        )
                        nop.engine = inst.engine
                        nop.sync_info = mybir.SyncInfo(on_wait=[w], on_update=[])
                        new_insts.append(nop)
                    inst.sync_info = mybir.SyncInfo(
                        on_wait=waits[-limit:],
                        on_update=list(si.on_update) if si.on_update else [],
                    )
                    n_split += 1
                new_insts.append(inst)
            if len(new_insts) != len(bb.instructions):
                bb.instructions = new_insts
                assert len(bb.instructions) == len(new_insts)
    return n_split


def _gram_exp(nc, psum_g, opT, e_pool, z_pack, zoff, etag):
    """opT: one [128, 28, 128] bf16 transposed tile (pad rows zero).
    Returns E = exp(scale*gram) (4 x [128, 512] bf16); rowsum(E) goes to
    z_pack[:, zoff+cb] f32 via the fused ACT accumulator."""
    es = []
    for cb in range(N_CB):
        g = psum_g.tile([P, C], F32, tag="g")
        for k in range(N_KCH):
            nc.tensor.matmul(
                g[:],
                opT[:, N_KCH * cb + k, :],
                opT[:, k :: N_KCH, :],
                start=(k == 0),
                stop=(k == N_KCH - 1),
            )
        e = e_pool.tile([P, C], BF16, tag=etag)
        nc.scalar.activation(
            e[:],
            g[:],
            AF.Exp,
            scale=SCALE,
            accum_out=z_pack[:, zoff + cb : zoff + cb + 1],
        )
        es.append(e)
    return es


def build_kernel():
    _patch_tile_drain()
    nc = bass.Bass()
    x_ext = nc.declare_dram_parameter("x", [B_PER_CORE, C, HW], F32, isOutput=False)
    c_ext = nc.declare_dram_parameter(
        "condition", [B_PER_CORE, C, HW], F32, isOutput=False
    )
    out_ext = nc.declare_dram_parameter("out", [B_PER_CORE, C, HW], F32, isOutput=True)

    eye_dram = nc.inline_tensor(np.eye(P, dtype=ml_dtypes.bfloat16), name="eye128")

    with tile.TileContext(nc) as tc:
        with (
            tc.tile_pool(name="const", bufs=1) as const_pool,
            tc.tile_pool(name="xn", bufs=3) as xn_pool,
            tc.tile_pool(name="cs", bufs=2) as cs_pool,
            tc.tile_pool(name="ct", bufs=2) as ct_pool,
            tc.tile_pool(name="xT", bufs=3) as xT_pool,
            tc.tile_pool(name="csT", bufs=3) as csT_pool,
            tc.tile_pool(name="E", bufs=12) as e_pool,
            tc.tile_pool(name="W", bufs=2) as w_pool,
            tc.tile_pool(name="WT", bufs=3) as wT_pool,
            tc.tile_pool(name="Wt", bufs=2) as t_pool,
            tc.tile_pool(name="zr", bufs=6) as zr_pool,
            tc.tile_pool(name="outs", bufs=2) as out_pool,
            tc.tile_pool(name="psum_g", bufs=2, space="PSUM") as psum_g,
            tc.tile_pool(name="psum_u", bufs=2, space="PSUM") as psum_u,
            tc.tile_pool(name="psum_t", bufs=2, space="PSUM") as psum_t,
        ):
            eye = const_pool.tile([P, P], BF16)
            nc.sync.dma_start(eye[:], eye_dram[:])

            staged = {}
            grams = {}
            applies = {}

            def stage(b):
                """cast-loads + sigmoid-via-tanh + SBUF->SBUF xbar transposes."""
                xTb = xT_pool.tile([P, N_CB * N_KCH, P], BF16, tag="xT")
                csTb = csT_pool.tile([P, N_CB * N_KCH, P], BF16, tag="csT")
                xnb = xn_pool.tile([P, N_CB, HWP], BF16, tag="xn")
                nc.gpsimd.memset(xnb[:, :, HW:], 0.0)
                nc.gpsimd.dma_start(
                    xnb[:, :, :HW],
                    x_ext[b].rearrange("(k p) n -> p k n", p=P),
                )
                for cb in range(N_CB):
                    nc.sync.dma_start_transpose(
                        xTb[:, N_KCH * cb : N_KCH * (cb + 1), :], xnb[:, cb, :]
                    )
                csb = cs_pool.tile([P, N_CB, HWP], BF16, tag="cs")
                ct = ct_pool.tile([P, N_CB, HW], BF16, tag="ct")
                nc.gpsimd.memset(csb[:, :, HW:], 0.0)
                nc.gpsimd.dma_start(
                    csb[:, :, :HW], c_ext[b].rearrange("(k p) n -> p k n", p=P)
                )
                # sigmoid(z) = 0.5 * tanh(z/2) + 0.5 (same ACT set as exp)
                nc.scalar.activation(ct[:], csb[:, :, :HW], AF.Tanh, scale=0.5)
                nc.vector.tensor_scalar(csb[:, :, :HW], ct[:], 0.5, 0.5, MUL, ADD)
                for cb in range(N_CB):
                    nc.sync.dma_start_transpose(
                        csTb[:, N_KCH * cb : N_KCH * (cb + 1), :], csb[:, cb, :]
                    )
                staged[b] = (xnb, xTb, csTb)

            def gram_core(b):
                xnb, xTb, csTb = staged.pop(b)
                z_pack = zr_pool.tile([P, 8], F32, tag="z")
                E1 = _gram_exp(nc, psum_g, xTb, e_pool, z_pack, 0, "e1")
                E2 = _gram_exp(nc, psum_g, csTb, e_pool, z_pack, 4, "e2")
                grams[b] = (xnb, z_pack, E1, E2)

            def combine(b):
                """W[c,d] = r1[c]*E1[c,d] + r2[c]*E2[c,d] (+I on diag block),
                built per-partition in [c-part, d-free] layout, then 16
                TensorE block transposes into the [d-part, c-free]
                stationary layout the apply needs."""
                xnb, z_pack, E1, E2 = grams.pop(b)
                r_pack = zr_pool.tile([P, 8], F32, tag="r")
                nc.vector.reciprocal(r_pack[:], z_pack[:])
                wbig = w_pool.tile([P, N_CB, C], BF16, tag="w")
                for a in range(N_CB):
                    t1 = t_pool.tile([P, C], BF16, tag="t1")
                    nc.vector.tensor_scalar(
                        t1[:], E1[a][:], r_pack[:, a : a + 1], None, MUL
                    )
                    nc.vector.scalar_tensor_tensor(
                        wbig[:, a, :],
                        E2[a][:],
                        r_pack[:, 4 + a : 5 + a],
                        t1[:],
                        MUL,
                        ADD,
                    )
                    nc.vector.tensor_add(
                        wbig[:, a, a * P : (a + 1) * P],
                        wbig[:, a, a * P : (a + 1) * P],
                        eye[:],
                    )
                WT = wT_pool.tile([P, N_CB * N_CB, P], BF16, tag="wT")
                for a in range(N_CB):
                    for e in range(N_CB):
                        tp = psum_t.tile([P, P], BF16, tag="tp")
                        nc.tensor.transpose(
                            tp[:], wbig[:, a, e * P : (e + 1) * P], eye[:]
                        )
                        if (a + e) % 2 == 0:
                            nc.scalar.activation(
                                WT[:, N_CB * a + e, :], tp[:], AF.Copy
                            )
                        else:
                            nc.vector.tensor_copy(WT[:, N_CB * a + e, :], tp[:])
                applies[b] = (xnb, WT)

            def apply_stage(b):
                xnb, WT = applies.pop(b)
                obig = out_pool.tile([P, N_CB, HW], BF16, tag="o")
                for cb in range(N_CB):
                    u = psum_u.tile([P, 1024], F32, tag="u")
                    for n0, nw in APPLY_NSPLIT:
                        for k in range(N_CB):
                            nc.tensor.matmul(
                                u[:, n0 : n0 + nw],
                                WT[:, N_CB * cb + k, :],
                                xnb[:, k, n0 : n0 + nw],
                                start=(k == 0),
                                stop=(k == N_CB - 1),
                            )
                    if cb % 2 == 0:
                        nc.scalar.activation(obig[:, cb, :], u[:, :HW], AF.Copy)
                    else:
                        nc.vector.tensor_copy(obig[:, cb, :], u[:, :HW])
                nc.gpsimd.dma_start(
                    out_ext[b].rearrange("(k p) n -> p k n", p=P), obig[:]
                )

            stage(0)
            stage(1)
            gram_core(0)
            combine(0)
            for b in range(B_PER_CORE):
                if b + 2 < B_PER_CORE:
                    stage(b + 2)
                if b + 1 < B_PER_CORE:
                    gram_core(b + 1)
                apply_stage(b)
                if b + 1 < B_PER_CORE:
                    combine(b + 1)
    n = _split_multi_waits(nc)
    print(f"[kernel] split {n} multi-wait instructions")
    return nc


_NC_CACHE = None


def kernel(x: np.ndarray, condition: np.ndarray, _trace: bool = False):
    """Full inputs [64, 512, 28, 28] f32 -> full output [64, 512, 784] f32."""
    global _NC_CACHE
    B = x.shape[0]
    xf = np.ascontiguousarray(x.reshape(B, C, HW), dtype=np.float32)
    cf = np.ascontiguousarray(condition.reshape(B, C, HW), dtype=np.float32)

    if _NC_CACHE is None:
        _NC_CACHE = build_kernel()
    nc = _NC_CACHE

    in_maps = [
        {
            "x": xf[i * B_PER_CORE : (i + 1) * B_PER_CORE],
            "condition": cf[i * B_PER_CORE : (i + 1) * B_PER_CORE],
        }
        for i in range(N_CORES)
    ]
    res = run_bass_kernel_spmd(nc, in_maps, core_ids=list(range(N_CORES)), trace=_trace)
    out = np.concatenate([res.results[i]["out"] for i in range(N_CORES)], axis=0)
    kernel.last_result = res
    return out


# revision 8
# speedup vs baseline: 1.1519x; 1.0069x over previous
"""Trainium2 Bass kernel for nn_Attention_75342316306884.

Per-batch channel-channel attention:
  xf = x.reshape(B, C, HW); cf = condition.reshape(B, C, HW)
  w1 = softmax(xf @ xf^T * HW^-0.5); w2 = softmax(sig(cf) @ sig(cf)^T * HW^-0.5)
  out = xf + (w1 + w2) @ xf          -> [B, C, HW] float32

Sharding: pure data parallel, batch dim 64 -> 8 cores x 8 batches.

Per-core pipeline, software-skewed two batches ahead (emission order
stage(b+2), gram(b+1), apply(b)):
  stage: one cast-DMA per tensor (f32 HBM -> bf16 [128, 4, 896] SBUF tile;
    pad cols zeroed by a gpsimd memset so gram chunks can run K=128 over
    the pad); condition -> sigmoid via tanh; ONE SBUF->SBUF xbar
    DMA-transpose per tensor into a [128, 28, 128] bf16 tile.
  gram: two 512x512 grams on TensorE (bf16, f32 PSUM accumulate), 7 full
    K=128 chunk matmuls each (zero pad contributes nothing); ACT exp with
    fused per-row accumulation into one packed z [128, 8] tile.
  combine (the main trick vs the two-chain baseline):
    r = 1/z on DVE; the merged attention matrix
    W[c, d] = r1[c]*E1[c,d] + r2[c]*E2[c,d] (+ I for the residual) is
    built in the native [c-part, d-free] layout with per-partition
    scalar ops (2 DVE ops per c-block: tensor_scalar mult +
    scalar_tensor_tensor fused mult-add), then 16 TensorE [128,128] block
    transposes (+ ACT/DVE PSUM->SBUF copies) turn it into the
    [d-part, c-free] stationary operand the apply needs.  The x/cs xbar
    DMA-transposes are split into 4 per-c-block instructions each so
    their descriptor streams interleave across DMA queues instead of
    serializing behind one another on the sync engine.
  apply: ONE matmul chain per c-block (4 K=128 chunks x 2 n-splits) into a
    2-bank [128, 1024] PSUM tile; epilogue is a single PSUM->SBUF bf16
    copy (alternating ACT/DVE) and one cast-DMA per batch back to f32 HBM.

vs. the two-chain version this halves apply TensorE work (no separate
E2 chain, no diag-residual chain, no per-chain scale/add epilogue) at the
cost of ~12 cheap DVE/gpsimd ops per batch.  exp without max-subtraction
is safe: logits bounded by ~|x|^2/28 ~ 35.
"""

import sys

import numpy as np

for _p in ("/opt/trn_rl_repo",):
    if _p not in sys.path:
        sys.path.append(_p)

import ml_dtypes

import concourse.bass as bass
import concourse.mybir as mybir
import concourse.tile as tile
from concourse.bass_utils import run_bass_kernel_spmd
from concourse.vector_clock import ScopedClock

F32 = mybir.dt.float32
BF16 = mybir.dt.bfloat16
AF = mybir.ActivationFunctionType
MUL = mybir.AluOpType.mult
ADD = mybir.AluOpType.add

N_CORES = 8
B_PER_CORE = 8
C = 512  # channels
HW = 784  # 28*28
HWP = 896  # padded to 7*128 for the xbar transpose
SCALE = float(HW) ** -0.5
P = 128
N_KCH = 7  # gram contraction chunks, all K=128 (pad zeroed)
N_CB = 4  # 512 / 128 c-blocks
APPLY_NSPLIT = ((0, 512), (512, 272))


def _patch_tile_drain():
    """walrus codegen in this toolchain rejects >1 sem-wait on one SP CTRL
    (drain/nop) instruction; spread the Tile end-of-context drain waits
    across several nops instead."""
    if getattr(tile.TileContext, "_drain_patched", False):
        return

    def _drain_and_barrier(self, tick_clock, wait_clock):
        absorber = self.nc.sync.nop()
        wait_clock.add_sem_waits(
            absorber.ins, ScopedClock({None: tick_clock.global_clock})
        )
        si = absorber.ins.sync_info
        waits = list(si.on_wait) if si is not None and si.on_wait else []
        if len(waits) > 1:
            absorber.ins.sync_info = mybir.SyncInfo(on_wait=waits[:1], on_update=[])
            for w in waits[1:]:
                n2 = self.nc.sync.nop()
                n2.ins.sync_info = mybir.SyncInfo(on_wait=[w], on_update=[])
        self.nc.sync.drain()
        self.nc.all_engine_barrier()
        assert self.sems is not None
        popped = self.nc._tile_sem_poison_stack.pop()
        assert popped is self._sem_poison
        self.nc.clear_and_free_semaphores(list(self.sems.allocated().values()))
        self.nc.all_engine_barrier()

    tile.TileContext._drain_and_barrier = _drain_and_barrier
    tile.TileContext._drain_patched = True


def _split_multi_waits(nc, limit=1):
    """This walrus build allows only `limit` sem-wait commands per
    instruction.  Hoist excess waits onto same-engine NoOps placed
    immediately before the instruction (per-engine program order makes
    this semantically identical)."""
    n_split = 0
    for f in nc.m.functions:
        for bb in f.blocks:
            new_insts = []
            for inst in bb.instructions:
                si = inst.sync_info
                waits = list(si.on_wait) if si is not None and si.on_wait else []
                if len(waits) > limit:
                    for j, w in enumerate(waits[:-limit]):
                        nop = mybir.InstNoOp(
                            name=f"{inst.name}-wsplit{j}", ins=[], outs=[]
                        )
                        nop.engine = inst.engine
                        nop.sync_info = mybir.SyncInfo(on_wait=[w], on_update=[])
                        new_insts.append(nop)
                    inst.sync_info = mybir.SyncInfo(
                        on_wait=waits[-limit:],
                        on_update=list(si.on_update) if si.on_update else [],
                    )
                    n_split += 1
                new_insts.append(inst)
            if len(new_insts) != len(bb.instructions):
                bb.instructions = new_insts
                assert len(bb.instructions) == len(new_insts)
    return n_split


def _gram_exp(nc, psum_g, opT, e_pool, z_pack, zoff, etag):
    """opT: one [128, 28, 128] bf16 transposed tile (pad rows zero).
    Returns E = exp(scale*gram) (4 x [128, 512] bf16); rowsum(E) goes to
    z_pack[:, zoff+cb] f32 via the fused ACT accumulator."""
    es = []
    for cb in range(N_CB):
        g = psum_g.tile([P, C], F32, tag="g")
        for k in range(N_KCH):
            nc.tensor.matmul(
                g[:],
                opT[:, N_KCH * cb + k, :],
                opT[:, k :: N_KCH, :],
                start=(k == 0),
                stop=(k == N_KCH - 1),
            )
        e = e_pool.tile([P, C], BF16, tag=etag)
        nc.scalar.activation(
            e[:],
            g[:],
            AF.Exp,
            scale=SCALE,
            accum_out=z_pack[:, zoff + cb : zoff + cb + 1],
        )
        es.append(e)
    return es


def build_kernel():
    _patch_tile_drain()
    nc = bass.Bass()
    x_ext = nc.declare_dram_parameter("x", [B_PER_CORE, C, HW], F32, isOutput=False)
    c_ext = nc.declare_dram_parameter(
        "condition", [B_PER_CORE, C, HW], F32, isOutput=False
    )
    out_ext = nc.declare_dram_parameter("out", [B_PER_CORE, C, HW], F32, isOutput=True)

    eye_dram = nc.inline_tensor(np.eye(P, dtype=ml_dtypes.bfloat16), name="eye128")

    with tile.TileContext(nc) as tc:
        with (
            tc.tile_pool(name="const", bufs=1) as const_pool,
            tc.tile_pool(name="xn", bufs=3) as xn_pool,
            tc.tile_pool(name="cs", bufs=2) as cs_pool,
            tc.tile_pool(name="ct", bufs=2) as ct_pool,
            tc.tile_pool(name="xT", bufs=3) as xT_pool,
            tc.tile_pool(name="csT", bufs=3) as csT_pool,
            tc.tile_pool(name="E", bufs=12) as e_pool,
            tc.tile_pool(name="W", bufs=2) as w_pool,
            tc.tile_pool(name="WT", bufs=3) as wT_pool,
            tc.tile_pool(name="Wt", bufs=2) as t_pool,
            tc.tile_pool(name="zr", bufs=6) as zr_pool,
            tc.tile_pool(name="outs", bufs=2) as out_pool,
            tc.tile_pool(name="psum_g", bufs=2, space="PSUM") as psum_g,
            tc.tile_pool(name="psum_u", bufs=2, space="PSUM") as psum_u,
            tc.tile_pool(name="psum_t", bufs=2, space="PSUM") as psum_t,
        ):
            eye = const_pool.tile([P, P], BF16)
            nc.sync.dma_start(eye[:], eye_dram[:])

            staged = {}
            ready = {}
            grams = {}
            applies = {}

            def stage_load(b):
                """cast-loads + x xbar transposes (emitted early in the
                round so the loads land well before their consumers)."""
                xTb = xT_pool.tile([P, N_CB * N_KCH, P], BF16, tag="xT")
                xnb = xn_pool.tile([P, N_CB, HWP], BF16, tag="xn")
                nc.gpsimd.memset(xnb[:, :, HW:], 0.0)
                nc.gpsimd.dma_start(
                    xnb[:, :, :HW],
                    x_ext[b].rearrange("(k p) n -> p k n", p=P),
                )
                for cb in range(N_CB):
                    nc.sync.dma_start_transpose(
                        xTb[:, N_KCH * cb : N_KCH * (cb + 1), :], xnb[:, cb, :]
                    )
                csb = cs_pool.tile([P, N_CB, HWP], BF16, tag="cs")
                nc.gpsimd.memset(csb[:, :, HW:], 0.0)
                nc.gpsimd.dma_start(
                    csb[:, :, :HW], c_ext[b].rearrange("(k p) n -> p k n", p=P)
                )
                staged[b] = (xnb, xTb, csb)

            def stage_cs(b):
                """sigmoid + cs xbar transposes.  Emitted LATE in the round
                (after the previous batch's exps) so the tanh never
                head-of-line-blocks the ACT queue while waiting for its
                load."""
                xnb, xTb, csb = staged.pop(b)
                csTb = csT_pool.tile([P, N_CB * N_KCH, P], BF16, tag="csT")
                ct = ct_pool.tile([P, N_CB, HW], BF16, tag="ct")
                # sigmoid(z) = 0.5 * tanh(z/2) + 0.5 (same ACT set as exp)
                nc.scalar.activation(ct[:], csb[:, :, :HW], AF.Tanh, scale=0.5)
                nc.vector.tensor_scalar(csb[:, :, :HW], ct[:], 0.5, 0.5, MUL, ADD)
                for cb in range(N_CB):
                    nc.sync.dma_start_transpose(
                        csTb[:, N_KCH * cb : N_KCH * (cb + 1), :], csb[:, cb, :]
                    )
                ready[b] = (xnb, xTb, csTb)

            def gram_core(b):
                xnb, xTb, csTb = ready.pop(b)
                z_pack = zr_pool.tile([P, 8], F32, tag="z")
                E1 = _gram_exp(nc, psum_g, xTb, e_pool, z_pack, 0, "e1")
                E2 = _gram_exp(nc, psum_g, csTb, e_pool, z_pack, 4, "e2")
                grams[b] = (xnb, z_pack, E1, E2)

            def combine(b):
                """W[c,d] = r1[c]*E1[c,d] + r2[c]*E2[c,d] (+I on diag block),
                built per-partition in [c-part, d-free] layout, then 16
                TensorE block transposes into the [d-part, c-free]
                stationary layout the apply needs."""
                xnb, z_pack, E1, E2 = grams.pop(b)
                r_pack = zr_pool.tile([P, 8], F32, tag="r")
                nc.vector.reciprocal(r_pack[:], z_pack[:])
                wbig = w_pool.tile([P, N_CB, C], BF16, tag="w")
                for a in range(N_CB):
                    t1 = t_pool.tile([P, C], BF16, tag="t1")
                    nc.vector.tensor_scalar(
                        t1[:], E1[a][:], r_pack[:, a : a + 1], None, MUL
                    )
                    nc.vector.scalar_tensor_tensor(
                        wbig[:, a, :],
                        E2[a][:],
                        r_pack[:, 4 + a : 5 + a],
                        t1[:],
                        MUL,
                        ADD,
                    )
                    nc.vector.tensor_add(
                        wbig[:, a, a * P : (a + 1) * P],
                        wbig[:, a, a * P : (a + 1) * P],
                        eye[:],
                    )
                WT = wT_pool.tile([P, N_CB * N_CB, P], BF16, tag="wT")
                for a in range(N_CB):
                    for e in range(N_CB):
                        tp = psum_t.tile([P, P], BF16, tag="tp")
                        nc.tensor.transpose(
                            tp[:], wbig[:, a, e * P : (e + 1) * P], eye[:]
                        )
                        nc.vector.tensor_copy(WT[:, N_CB * a + e, :], tp[:])
                applies[b] = (xnb, WT)

            def apply_stage(b):
                xnb, WT = applies.pop(b)
                obig = out_pool.tile([P, N_CB, HW], BF16, tag="o")
                for cb in range(N_CB):
                    u = psum_u.tile([P, 1024], F32, tag="u")
                    for n0, nw in APPLY_NSPLIT:
                        for k in range(N_CB):
                            nc.tensor.matmul(
                                u[:, n0 : n0 + nw],
                                WT[:, N_CB * cb + k, :],
                                xnb[:, k, n0 : n0 + nw],
                                start=(k == 0),
                                stop=(k == N_CB - 1),
                            )
                    if cb % 2 == 0:
                        nc.scalar.activation(obig[:, cb, :], u[:, :HW], AF.Copy)
                    else:
                        nc.vector.tensor_copy(obig[:, cb, :], u[:, :HW])
                nc.gpsimd.dma_start(
                    out_ext[b].rearrange("(k p) n -> p k n", p=P), obig[:]
                )

            stage_load(0)
            stage_cs(0)
            stage_load(1)
            stage_cs(1)
            gram_core(0)
            combine(0)
            for b in range(B_PER_CORE):
                if b + 2 < B_PER_CORE:
                    stage_load(b + 2)
                if b + 1 < B_PER_CORE:
                    gram_core(b + 1)
                apply_stage(b)
                if b + 2 < B_PER_CORE:
                    stage_cs(b + 2)
                if b + 1 < B_PER_CORE:
                    combine(b + 1)
    n = _split_multi_waits(nc)
    print(f"[kernel] split {n} multi-wait instructions")
    return nc


_NC_CACHE = None


def kernel(x: np.ndarray, condition: np.ndarray, _trace: bool = False):
    """Full inputs [64, 512, 28, 28] f32 -> full output [64, 512, 784] f32."""
    global _NC_CACHE
    B = x.shape[0]
    xf = np.ascontiguousarray(x.reshape(B, C, HW), dtype=np.float32)
    cf = np.ascontiguousarray(condition.reshape(B, C, HW), dtype=np.float32)

    if _NC_CACHE is None:
        _NC_CACHE = build_kernel()
    nc = _NC_CACHE

    in_maps = [
        {
            "x": xf[i * B_PER_CORE : (i + 1) * B_PER_CORE],
            "condition": cf[i * B_PER_CORE : (i + 1) * B_PER_CORE],
        }
        for i in range(N_CORES)
    ]
    res = run_bass_kernel_spmd(nc, in_maps, core_ids=list(range(N_CORES)), trace=_trace)
    out = np.concatenate([res.results[i]["out"] for i in range(N_CORES)], axis=0)
    kernel.last_result = res
    return out
